# revision 8
# baseline (speedup 1.0000x reference)
"""GCN layer kernel for nn_GCNLayer_20547123544324 on 8 Trainium2 NeuronCores.

Computes a PyG-style GCNConv:
    out = D^-1/2 (A + I) D^-1/2 (x @ W) + b
       == (D^-1/2 (A + I) D^-1/2 x) @ W + b        (associativity)

Node-partitioned (per the sharding hint): 12500 dst nodes per core; edges
(incl. self-loops) bucketed by 128-node dst windows, padded to a static
B=19 blocks of 128 edges per window (uniform SPMD program).

Device pipeline per core:
  - per 128-edge block: indirect-DMA row gather x[src] (bf16 rows padded
    to 256B stride; one row per partition — the canonical form this
    hardware's SWDGE path implements correctly)
  - per window: one-hot scatter matrices S'[e, j] = norm_e * (dst_e == j)
    built by DVE iota-compare then norm scale (norm folded here keeps the
    per-block DVE work off the critical path)
  - PE: aggT[64 feat, 128 dst] += gathered_block.T @ S'_block, accumulated
    in PSUM over the window's 19 blocks
  - PE: out_win[128, 64] = aggT.T @ W  (aggT is already in lhsT layout)
  - DVE bias add, DMA out.

Self-contained: hardcoded N=100000, E=1600000, D=64, 8 cores.
"""
import numpy as np
import ml_dtypes

import jax
from jax.sharding import Mesh, PartitionSpec
from jax.experimental.shard_map import shard_map

import concourse.bass as bass
import concourse.mybir as mybir
import concourse.tile as tile
from concourse import bacc
from concourse.bass2jax import _bass_exec_p, install_neuronx_cc_hook, \
    partition_id_tensor

N = 100000
E = 1600000
D = 64
M = 8                 # cores
NPC = N // M          # 12500 nodes per core
P = 128
NWIN = -(-NPC // P)   # 98 windows (last holds 84 nodes)
B = 19                # blocks of 128 edges per window (static cap)
NBLK = NWIN * B       # 1862

BF16 = mybir.dt.bfloat16
F32 = mybir.dt.float32
I32 = mybir.dt.int32

_cache = {}


NQUEUE = 4


def build_program(reps: int = 1):
    nc = bacc.Bacc("TRN2", target_bir_lowering=False, debug=False,
                   num_devices=M, num_swdge_queues=NQUEUE)

    t_x = nc.dram_tensor("xbf", [N, 2 * D], BF16, kind="ExternalInput").ap()
    t_idx = nc.dram_tensor("idxm", [P, NBLK], I32, kind="ExternalInput").ap()
    t_dst = nc.dram_tensor("dstm", [P, NBLK], BF16, kind="ExternalInput").ap()
    t_nrm = nc.dram_tensor("nrmm", [P, NBLK], BF16, kind="ExternalInput").ap()
    t_iota = nc.dram_tensor("iota", [P, P], BF16, kind="ExternalInput").ap()
    t_W = nc.dram_tensor("Wt", [D, D], BF16, kind="ExternalInput").ap()
    t_bias = nc.dram_tensor("biasr", [P, D], F32, kind="ExternalInput").ap()
    t_out = nc.dram_tensor("out", [NPC, D], F32, kind="ExternalOutput").ap()

    with tile.TileContext(nc) as tc:
        with (
            tc.tile_pool(name="const", bufs=1) as constp,
            tc.tile_pool(name="meta", bufs=1) as metap,
            tc.tile_pool(name="gath", bufs=12) as gathp,
            tc.tile_pool(name="onehot", bufs=3) as onep,
            tc.tile_pool(name="sb", bufs=4) as sbp,
            tc.tile_pool(name="agg", bufs=4, space="PSUM") as aggp,
            tc.tile_pool(name="proj", bufs=2, space="PSUM") as projp,
        ):
            iota_sb = constp.tile([P, P], BF16)
            nc.sync.dma_start(out=iota_sb[:], in_=t_iota[:])
            W_sb = constp.tile([D, D], BF16)
            nc.sync.dma_start(out=W_sb[:], in_=t_W[:])
            bias_sb = constp.tile([P, D], F32)
            nc.sync.dma_start(out=bias_sb[:], in_=t_bias[:])
            idx_sb = metap.tile([P, NBLK], I32)
            nc.sync.dma_start(out=idx_sb[:], in_=t_idx[:])
            dst_sb = metap.tile([P, NBLK], BF16)
            nc.sync.dma_start(out=dst_sb[:], in_=t_dst[:])
            nrm_sb = metap.tile([P, NBLK], BF16)
            nc.sync.dma_start(out=nrm_sb[:], in_=t_nrm[:])

            for _rep in range(reps):
                for w in range(NWIN):
                    oh = onep.tile([P, B, P], BF16, tag="oh")
                    nc.vector.tensor_tensor(
                        out=oh[:],
                        in0=dst_sb[:, w * B:(w + 1) * B].unsqueeze(2)
                            .to_broadcast([P, B, P]),
                        in1=iota_sb[:].unsqueeze(1).to_broadcast([P, B, P]),
                        op=mybir.AluOpType.is_equal,
                    )
                    nc.vector.tensor_tensor(
                        out=oh[:],
                        in0=oh[:],
                        in1=nrm_sb[:, w * B:(w + 1) * B].unsqueeze(2)
                            .to_broadcast([P, B, P]),
                        op=mybir.AluOpType.mult,
                    )
                    aggT = aggp.tile([D, P], F32, space="PSUM", tag="agg")
                    for j in range(B):
                        blk = w * B + j
                        g = gathp.tile([P, D], BF16, tag="g")
                        inst = nc.gpsimd.indirect_dma_start(
                            out=g[:], out_offset=None, in_=t_x[:],
                            in_offset=bass.IndirectOffsetOnAxis(
                                ap=idx_sb[:, blk:blk + 1], axis=0))
                        if blk % NQUEUE:
                            inst.queue = f"qPoolDynamic{blk % NQUEUE}"
                        nc.tensor.matmul(
                            out=aggT[:],
                            lhsT=g[:],
                            rhs=oh[:, j, :],
                            start=(j == 0),
                            stop=(j == B - 1),
                        )
                    aggT_sb = sbp.tile([D, P], BF16, tag="aggsb")
                    nc.vector.tensor_copy(out=aggT_sb[:], in_=aggT[:])
                    pr = projp.tile([P, D], F32, space="PSUM", tag="pr")
                    nc.tensor.matmul(out=pr[:], lhsT=aggT_sb[:], rhs=W_sb[:],
                                     start=True, stop=True)
                    out_sb = sbp.tile([P, D], F32, tag="outsb")
                    nc.vector.tensor_add(out=out_sb[:], in0=pr[:],
                                         in1=bias_sb[:])
                    rows = min(P, NPC - w * P)
                    nc.sync.dma_start(out=t_out[w * P:w * P + rows, :],
                                      in_=out_sb[:rows, :])

    nc.compile()
    return nc


def _prep_inputs(x, edge_index, W, b):
    x = np.asarray(x, dtype=np.float32)
    W = np.asarray(W, dtype=np.float32)
    b = np.asarray(b, dtype=np.float32)
    ei = np.asarray(edge_index)
    src_e = ei[0].astype(np.int64)
    dst_e = ei[1].astype(np.int64)

    deg = (np.bincount(dst_e, minlength=N) + 1).astype(np.float32)
    dinv = (1.0 / np.sqrt(deg)).astype(np.float32)

    loop = np.arange(N, dtype=np.int64)
    src = np.concatenate([src_e, loop])
    dst = np.concatenate([dst_e, loop])
    norm = np.concatenate([dinv[src_e] * dinv[dst_e],
                           1.0 / deg]).astype(np.float32)

    core = dst // NPC
    ldst = dst - core * NPC
    w = ldst >> 7
    doff = (ldst & 127).astype(np.float32)

    gwin = core * NWIN + w
    order = np.argsort(gwin, kind="stable")
    gwin_s = gwin[order]
    nwin_tot = M * NWIN
    starts = np.searchsorted(gwin_s, np.arange(nwin_tot))
    ends = np.searchsorted(gwin_s, np.arange(nwin_tot), side="right")
    assert (ends - starts).max() <= B * P, f"overflow {(ends-starts).max()}"
    rank = np.empty_like(order)
    rank[order] = np.arange(len(order)) - starts[gwin_s]

    blk = w * B + (rank >> 7)          # per-core block id (window-major)
    lane = rank & 127

    bf = ml_dtypes.bfloat16
    idxm = np.zeros((M, P, NBLK), np.int32)
    dstm = np.zeros((M, P, NBLK), np.float32)
    nrmm = np.zeros((M, P, NBLK), np.float32)
    idxm[core, lane, blk] = src
    dstm[core, lane, blk] = doff
    nrmm[core, lane, blk] = norm

    x_pad = np.zeros((N, 2 * D), bf)
    x_pad[:, :D] = x.astype(bf)
    iota = np.tile(np.arange(P, dtype=np.float32), (P, 1)).astype(bf)
    W_bf = W.astype(bf)
    bias_rep = np.tile(b, (P, 1)).astype(np.float32)

    in_maps = []
    for c in range(M):
        in_maps.append({
            "xbf": x_pad,
            "idxm": idxm[c],
            "dstm": dstm[c].astype(bf),
            "nrmm": nrmm[c].astype(bf),
            "iota": iota,
            "Wt": W_bf,
            "biasr": bias_rep,
        })
    return in_maps


class SpmdRunner:
    """Cached-executable SPMD runner: jit the bass program once, reuse the
    compiled callable across calls (mirrors bass2jax.run_bass_via_pjrt's
    multi-core path, minus per-call re-jitting)."""

    def __init__(self, nc, n_cores=M):
        install_neuronx_cc_hook()
        self.nc = nc
        self.n_cores = n_cores
        assert nc.dbg_addr is None

        partition_name = (nc.partition_id_tensor.name
                          if nc.partition_id_tensor else None)
        in_names, out_names, out_avals, zero_outs = [], [], [], []
        for alloc in nc.m.functions[0].allocations:
            if not isinstance(alloc, mybir.MemoryLocationSet):
                continue
            name = alloc.memorylocations[0].name
            if alloc.kind == "ExternalInput":
                if name != partition_name:
                    in_names.append(name)
            elif alloc.kind == "ExternalOutput":
                shape = tuple(alloc.tensor_shape)
                dtype = mybir.dt.np(alloc.dtype)
                out_names.append(name)
                out_avals.append(jax.core.ShapedArray(shape, dtype))
                zero_outs.append(np.zeros(shape, dtype))
        self.in_names = list(in_names)
        self.out_names = out_names
        self.out_avals = out_avals
        self.zero_outs = zero_outs
        n_params = len(self.in_names)
        n_outs = len(out_avals)
        all_in_names = self.in_names + out_names
        if partition_name is not None:
            all_in_names.append(partition_name)

        def _body(*args):
            operands = list(args)
            if partition_name is not None:
                operands.append(partition_id_tensor())
            outs = _bass_exec_p.bind(
                *operands,
                out_avals=tuple(out_avals),
                in_names=tuple(all_in_names),
                out_names=tuple(out_names),
                lowering_input_output_aliases=(),
                sim_require_finite=True,
                sim_require_nnan=True,
                nc=nc,
            )
            return tuple(outs)

        devices = jax.devices()[:n_cores]
        assert len(devices) == n_cores
        self.mesh = Mesh(np.asarray(devices), ("core",))
        in_specs = (PartitionSpec("core"),) * (n_params + n_outs)
        out_specs = (PartitionSpec("core"),) * n_outs
        self.fn = jax.jit(shard_map(_body, mesh=self.mesh, in_specs=in_specs,
                                    out_specs=out_specs, check_rep=False),
                          keep_unused=True)
        self._dev_zeros = None

    def put_inputs(self, in_maps):
        n = self.n_cores
        concat = [np.concatenate([np.asarray(in_maps[c][name])
                                  for c in range(n)], axis=0)
                  for name in self.in_names]
        sharding = jax.sharding.NamedSharding(self.mesh, PartitionSpec("core"))
        return [jax.device_put(a, sharding) for a in concat]

    def run(self, dev_inputs):
        if self._dev_zeros is None:
            sharding = jax.sharding.NamedSharding(self.mesh,
                                                  PartitionSpec("core"))
            self._dev_zeros = [
                jax.device_put(
                    np.zeros((self.n_cores * z.shape[0], *z.shape[1:]),
                             z.dtype), sharding)
                for z in self.zero_outs]
        out = self.fn(*dev_inputs, *self._dev_zeros)
        jax.block_until_ready(out)
        return out

    def results(self, out_arrs):
        n = self.n_cores
        return [
            {name: np.asarray(out_arrs[i]).reshape(
                n, *self.out_avals[i].shape)[c]
             for i, name in enumerate(self.out_names)}
            for c in range(n)
        ]

    def __call__(self, in_maps):
        return self.results(self.run(self.put_inputs(in_maps)))


def kernel(x, edge_index, W, b):
    if "runner" not in _cache:
        _cache["runner"] = SpmdRunner(build_program(reps=1), M)
    r = _cache["runner"]
    in_maps = _prep_inputs(x, edge_index, W, b)
    res = r(in_maps)
    out = np.concatenate([rr["out"] for rr in res], axis=0)
    return out.astype(np.float32)
